# revision 27
# baseline (speedup 1.0000x reference)
"""Trainium2 Bass kernel for causal multi-head attention.

Problem: x[2, 2048, 1024], W_Q/W_K/W_V/W_O [1024, 1024], 16 heads, d_k=64,
causal softmax attention, fp32.

Sharding (8 cores): core c owns batch b=c//4 and head-group g=c%4 (4 heads,
256 cols of W_Q/K/V, 256 rows of W_O). Each core computes a full [S, D]
partial output (its heads' contribution through W_O); host sums the 4
partials per batch.

Device-side per core (all matmuls float32r = fp32 rounded to 11 mantissa
bits, full PE speed at free-dim>=256):
  1. QT/KT/VT = (x @ W)^T via matmuls with W chunks stationary, x^T moving
     (x^T prepared host-side).
  2. V' tiles [128, 65]: V natural layout (PE transpose of VT) + ones column
     (so attn@V also produces softmax denominators for free).
  3. Per (head, q-tile of 512): scores^T[k, q] = K^T-chunk.T @ Q^T (k on
     partitions -> no transpose of probs needed), exp on ScalarE with
     scale=1/8 folded in, causal triangle masked by elementwise multiply,
     attnV: A[65, 512] += V'[kc].T @ E[kc] accumulating over k-chunks.
     Row 64 of A = sum_k exp = softmax denominator.
  4. Normalize: reciprocal_approx_fast on denom row, broadcast via
     ones-matmul, multiply -> NT_h [64, S] normalized out^T per head.
  5. partial^T[e, s] = sum_h W_O[h-rows].T-chunk @ NT_h -> DMA out.
"""

import numpy as np
from contextlib import ExitStack

import concourse.bass as bass
import concourse.tile as tile
from concourse import bacc, mybir
from concourse.bass_utils import run_bass_kernel_spmd

dt = mybir.dt
AF = mybir.ActivationFunctionType

B, S, D, NH, DK = 2, 2048, 1024, 16, 64
NCORES = 8
HPC = 4            # heads per core
CW = HPC * DK      # 256 per-core col width of W_Q/K/V (rows of W_O)
QT_W = 512         # q-tile width
KC_W = 128         # k-chunk width
NQT = S // QT_W    # 4
NKC = S // KC_W    # 16
NDC = D // 128     # 8 contraction chunks for projections
NEC = D // 128     # 8 output-row chunks for W_O stage


def _round_f32r(a: np.ndarray) -> np.ndarray:
    """Round fp32 to f32r (11 mantissa bits, round-half-up) host-side."""
    b = np.ascontiguousarray(a, dtype=np.float32).view(np.uint32)
    b = (b + np.uint32(0x800)) & np.uint32(0xFFFFF000)
    return b.view(np.float32)


def build(debug=False):
    nc = bacc.Bacc("TRN2", target_bir_lowering=False, debug=False,
                   num_devices=NCORES)

    xt_d = nc.dram_tensor("xt", [D, S], dt.float32r, kind="ExternalInput").ap()
    wq_d = nc.dram_tensor("wq", [D, CW], dt.float32r, kind="ExternalInput").ap()
    wk_d = nc.dram_tensor("wk", [D, CW], dt.float32r, kind="ExternalInput").ap()
    wv_d = nc.dram_tensor("wv", [D, CW], dt.float32r, kind="ExternalInput").ap()
    wo_d = nc.dram_tensor("wo", [CW, D], dt.float16, kind="ExternalInput").ap()
    on_d = nc.dram_tensor("ones", [DK + 1, DK], dt.float16, kind="ExternalInput").ap()
    tri_d = nc.dram_tensor("tri", [KC_W, KC_W], dt.float16, kind="ExternalInput").ap()
    vo_d = nc.dram_tensor("vones", [128, NKC * (DK + 1)], dt.float16,
                          kind="ExternalInput").ap()
    o_d = [nc.dram_tensor(f"o{i}", [D, S], dt.float32, kind="ExternalOutput").ap()
           for i in range(2)]
    dbg = {}
    if debug:
        for nm, shp, dty in (("dbg_qt", [128, S], dt.float16),
                             ("dbg_kt", [128, S], dt.float16),
                             ("dbg_vp", [128, NKC * (DK + 1)], dt.float16),
                             ("dbg_e", [128, 2 * QT_W], dt.float16),
                             ("dbg_os", [DK + 1, S], dt.float32),
                             ("dbg_rh", [DK + 1, S], dt.float16),
                             ("dbg_nt", [DK, S], dt.float16)):
            dbg[nm] = nc.dram_tensor(nm, shp, dty, kind="ExternalOutput").ap()

    ts = bass.ts

    with tile.TileContext(nc) as tc, ExitStack() as top:
        # ---- whole-kernel pools ----
        p_const = top.enter_context(tc.tile_pool(name="const", bufs=2))
        p_wo = top.enter_context(tc.tile_pool(name="wo", bufs=HPC))
        p_qt = top.enter_context(tc.tile_pool(name="qt", bufs=2))
        p_kt = top.enter_context(tc.tile_pool(name="kt", bufs=2))
        p_vp = top.enter_context(tc.tile_pool(name="vp", bufs=HPC))

        ones = p_const.tile([DK + 1, DK], dt.float16, name="ones", tag="ones")
        nc.scalar.dma_start(out=ones[:], in_=on_d[:])
        tri = p_const.tile([KC_W, KC_W], dt.float16, name="tri", tag="tri")
        nc.scalar.dma_start(out=tri[:], in_=tri_d[:])

        wo_sb = []
        for h in range(HPC):
            t = p_wo.tile([DK, D], dt.float16, name="wo", tag="wo")
            nc.scalar.dma_start(out=t[:], in_=wo_d[ts(h, DK), :])
            wo_sb.append(t)

        qt_sb = [p_qt.tile([128, S], dt.float16, name="qt", tag="qt") for _ in range(2)]
        kt_sb = [p_kt.tile([128, S], dt.float16, name="kt", tag="kt") for _ in range(2)]
        vp_sb = [p_vp.tile([128, NKC * (DK + 1)], dt.float16, name="vp", tag="vp")
                 for _ in range(HPC)]

        p_nt = top.enter_context(tc.tile_pool(name="nt", bufs=HPC))
        p_oc = top.enter_context(tc.tile_pool(name="oc", bufs=4))
        p_xt = top.enter_context(tc.tile_pool(name="xt", bufs=NDC))
        p_wv = top.enter_context(tc.tile_pool(name="wv", bufs=NDC))
        es_qk = ExitStack()   # closed after Q/K projections
        p_wqk = es_qk.enter_context(tc.tile_pool(name="wqk", bufs=2 * NDC))
        p_pp = es_qk.enter_context(tc.tile_pool(name="pp", bufs=6, space="PSUM"))

        w_sb = {}
        for mat, wd in (("q", wq_d), ("k", wk_d)):
            w_sb[mat] = []
            for dc in range(NDC):
                t = p_wqk.tile([128, CW], dt.float32r, name="wqk", tag="wqk")
                nc.gpsimd.dma_start(out=t[:], in_=wd[ts(dc, 128), :])
                w_sb[mat].append(t)
        w_sb["v"] = []
        for dc in range(NDC):
            t = p_wv.tile([128, CW], dt.float32r, name="wv", tag="wv")
            nc.gpsimd.dma_start(out=t[:], in_=wv_d[ts(dc, 128), :])
            w_sb["v"].append(t)

        # xt loaded dc-major (matches first consumption order)
        xt_sb = [p_xt.tile([128, S], dt.float32r, name="xt", tag="xt")
                 for _ in range(NDC)]
        for st in range(NQT):
            for dc in range(NDC):
                nc.sync.dma_start(out=xt_sb[dc][:, ts(st, QT_W)],
                                  in_=xt_d[ts(dc, 128), ts(st, QT_W)])

        for h in range(HPC):
            nc.scalar.dma_start(out=vp_sb[h][:], in_=vo_d[:])

        # ============ Q^T / K^T projections ============
        dests = {"q": qt_sb, "k": kt_sb}
        for st in range(NQT):
            for mat in ("q", "k"):
                for pg in range(2):
                    pp = p_pp.tile([128, QT_W], dt.float32, name="pp", tag="pp")
                    for dc in range(NDC):
                        nc.tensor.matmul(
                            pp[:],
                            w_sb[mat][dc][:, ts(pg, 128)],
                            xt_sb[dc][:, ts(st, QT_W)],
                            start=(dc == 0), stop=(dc == NDC - 1),
                        )
                    dst = dests[mat][pg][:, ts(st, QT_W)]
                    if mat == "q":
                        nc.scalar.copy(dst, pp[:])
                    else:
                        nc.vector.tensor_copy(dst, pp[:])
        if debug:
            nc.sync.dma_start(out=dbg["dbg_qt"][:], in_=qt_sb[0][:])
            nc.sync.dma_start(out=dbg["dbg_kt"][:], in_=kt_sb[0][:])
        es_qk.close()

        # ============ attention + V-nat + W_O, fully interleaved ============
        with tc.tile_pool(name="e", bufs=5) as p_e, \
             tc.tile_pool(name="os", bufs=2) as p_os, \
             tc.tile_pool(name="rc", bufs=2) as p_rc, \
             tc.tile_pool(name="rh", bufs=2) as p_rh, \
             tc.tile_pool(name="s", bufs=2, space="PSUM") as p_s, \
             tc.tile_pool(name="a", bufs=2, space="PSUM") as p_a, \
             tc.tile_pool(name="pt", bufs=2, space="PSUM") as p_pt:

            def vnat_chunk(sc):
                """V rows [128sc, 128sc+128) for all heads, in natural
                layout, via regular matmuls (x^T chunk stationary)."""
                pv = p_pt.tile([128, CW], dt.float32, name="pv", tag="pt")
                for dc in range(NDC):
                    nc.tensor.matmul(
                        pv[:, 0:CW],
                        xt_sb[dc][:, ts(sc, KC_W)],
                        w_sb["v"][dc][:],
                        start=(dc == 0), stop=(dc == NDC - 1),
                    )
                for h in range(HPC):
                    dst = vp_sb[h][:, sc * (DK + 1):sc * (DK + 1) + DK]
                    nc.vector.tensor_copy(dst, pv[:, ts(h, DK)])

            def scores_exp(h, qt, g2):
                """2-kc scores + exp for (head, q-tile, group) -> E tile."""
                pg, e = h // 2, h % 2
                prow = slice(e * DK, (e + 1) * DK)
                kcs = [2 * g2, 2 * g2 + 1]
                s_ps = p_s.tile([128, 2 * QT_W], dt.float32, name="s", tag="s")
                e_sb = p_e.tile([128, 2 * QT_W], dt.float16, name="e", tag="e")
                for j, kc in enumerate(kcs):
                    nc.tensor.matmul(
                        s_ps[:, ts(j, QT_W)],
                        kt_sb[pg][prow, ts(kc, KC_W)],
                        qt_sb[pg][prow, ts(qt, QT_W)],
                        start=True, stop=True,
                    )
                # one exp per group; non-causal cols of diagonal blocks are
                # never read by attnv (sub-span matmuls) except the 128-wide
                # triangle, masked explicitly
                nc.scalar.activation(e_sb[:], s_ps[:], AF.Exp, scale=0.125)
                for j, kc in enumerate(kcs):
                    r = kc - 4 * qt
                    if r >= 0:
                        lo = j * QT_W + r * KC_W
                        nc.vector.tensor_mul(
                            e_sb[:, lo:lo + KC_W],
                            e_sb[:, lo:lo + KC_W],
                            tri[:],
                        )
                return e_sb

            def attnv(h, qt, g2, e_sb, a_ps, nkc):
                for j, kc in enumerate([2 * g2, 2 * g2 + 1]):
                    r = kc - 4 * qt
                    first = (kc == 0)
                    last = (kc == nkc - 1)
                    if r > 0:
                        lo_q = r * KC_W
                        nc.tensor.matmul(
                            a_ps[:, lo_q:QT_W],
                            vp_sb[h][:, kc * (DK + 1):(kc + 1) * (DK + 1)],
                            e_sb[:, j * QT_W + lo_q:(j + 1) * QT_W],
                            start=False, stop=last,
                        )
                    else:
                        nc.tensor.matmul(
                            a_ps[:],
                            vp_sb[h][:, kc * (DK + 1):(kc + 1) * (DK + 1)],
                            e_sb[:, ts(j, QT_W)],
                            start=first, stop=last,
                        )

            def attn_qt_pair(h0, h1, qt, os0, os1):
                """Both heads' (qt) units, group-software-pipelined."""
                nkc = 4 * (qt + 1)
                ngr = nkc // 2
                a0 = p_a.tile([DK + 1, QT_W], dt.float32, name="a0", tag="a")
                a1 = p_a.tile([DK + 1, QT_W], dt.float32, name="a1", tag="a")
                prev = None
                for g2 in range(ngr):
                    e0 = scores_exp(h0, qt, g2)
                    e1 = scores_exp(h1, qt, g2)
                    if prev is not None:
                        attnv(h0, qt, g2 - 1, prev[0], a0, nkc)
                        attnv(h1, qt, g2 - 1, prev[1], a1, nkc)
                    prev = (e0, e1)
                attnv(h0, qt, ngr - 1, prev[0], a0, nkc)
                attnv(h1, qt, ngr - 1, prev[1], a1, nkc)
                nc.vector.tensor_copy(os0[:, ts(qt, QT_W)], a0[:])
                nc.vector.tensor_copy(os1[:, ts(qt, QT_W)], a1[:])
                if debug and h0 == 0 and qt == 3:
                    nc.sync.dma_start(out=dbg["dbg_os"][:], in_=os0[:])

            def normalize_qt(h, os_h, qt):
                # NB: reciprocal_approx_fast silently misbehaves on
                # partition-sliced APs on HW -- keep full partition range
                # (free-dim slicing is fine).
                rc = p_rc.tile([DK + 1, QT_W], dt.float32, name="rc", tag="rc")
                rh = p_rh.tile([DK + 1, QT_W], dt.float16, name="rh", tag="rh")
                nc.vector.reciprocal_approx_fast(
                    out=rc[:], in_=os_h[:, ts(qt, QT_W)])
                nc.vector.tensor_copy(rh[DK:DK + 1, :], rc[DK:DK + 1, :])
                bc = p_pt.tile([DK, QT_W], dt.float32, name="bc", tag="pt")
                nc.tensor.matmul(
                    bc[:], ones[DK:DK + 1, :], rh[DK:DK + 1, :],
                    start=True, stop=True,
                )
                nc.vector.tensor_mul(
                    nt_sb[h][:, ts(qt, QT_W)],
                    os_h[0:DK, ts(qt, QT_W)],
                    bc[:],
                )

            def wo_sth(hp, ec, sth):
                """partial^T for head pair hp, rows chunk ec, st half sth."""
                pt = [p_pt.tile([128, QT_W], dt.float32, name="pt", tag="pt")
                      for _ in range(2)]
                for h in (2 * hp, 2 * hp + 1):
                    for st in (2 * sth, 2 * sth + 1):
                        nc.tensor.matmul(
                            pt[st - 2 * sth][:],
                            wo_sb[h][:, ts(ec, 128)],
                            nt_sb[h][:, ts(st, QT_W)],
                            start=(h == 2 * hp), stop=(h == 2 * hp + 1),
                        )
                for st in (2 * sth, 2 * sth + 1):
                    oc = p_oc.tile([128, QT_W], dt.float32,
                                   name="oc", tag="oc")
                    if st % 2 == 0:
                        nc.vector.tensor_copy(oc[:], pt[st - 2 * sth][:])
                    else:
                        nc.scalar.copy(oc[:], pt[st - 2 * sth][:])
                    nc.sync.dma_start(
                        out=o_d[hp][ts(ec, 128), ts(st, QT_W)],
                        in_=oc[:])

            nt_sb = [p_nt.tile([DK, S], dt.float16, name="nt", tag="nt")
                     for _ in range(HPC)]
            # pair 0: attention with vnat interleaved as dense warm PE work
            os_t = {}
            for hp in range(HPC // 2):
                os_t[2 * hp] = p_os.tile([DK + 1, S], dt.float32,
                                         name="os0", tag="os")
                os_t[2 * hp + 1] = p_os.tile([DK + 1, S], dt.float32,
                                             name="os1", tag="os")
            for qt in range(NQT):
                for sc in range(4 * qt, 4 * qt + 4):
                    vnat_chunk(sc)
                attn_qt_pair(0, 1, qt, os_t[0], os_t[1])
            if debug:
                nc.sync.dma_start(out=dbg["dbg_vp"][:], in_=vp_sb[0][:])
            # pair 1: attention; pair-0 normalize + W_O interleaved so the
            # PE never idles long enough for HAM to re-throttle
            for qt in range(NQT):
                attn_qt_pair(2, 3, qt, os_t[2], os_t[3])
                if qt == 0:
                    for qt2 in range(NQT):
                        normalize_qt(0, os_t[0], qt2)
                        normalize_qt(1, os_t[1], qt2)
                    if debug:
                        nc.sync.dma_start(out=dbg["dbg_nt"][:], in_=nt_sb[0][:])
                elif qt < 3:
                    wo_sth(0, qt - 1, 0)
                    wo_sth(0, qt - 1, 1)
                else:
                    for ec in (2, 3):
                        wo_sth(0, ec, 0)
                        wo_sth(0, ec, 1)
            # tail
            for qt in range(NQT):
                normalize_qt(2, os_t[2], qt)
            for ec in (4, 5):
                wo_sth(0, ec, 0)
                wo_sth(0, ec, 1)
            for qt in range(NQT):
                normalize_qt(3, os_t[3], qt)
            for ec in (6, 7):
                wo_sth(0, ec, 0)
                wo_sth(0, ec, 1)
            for ec in range(NEC):
                wo_sth(1, ec, 0)
                wo_sth(1, ec, 1)

    nc.compile()
    return nc


_NC = None


def _get_nc():
    global _NC
    if _NC is None:
        _NC = build()
    return _NC


def make_in_maps(x, W_Q, W_K, W_V, W_O):
    x = np.asarray(x, np.float32)
    W_Q, W_K, W_V, W_O = (np.asarray(w, np.float32) for w in (W_Q, W_K, W_V, W_O))
    ones = np.ones((DK + 1, DK), np.float16)
    tri = (np.arange(KC_W)[:, None] <= np.arange(KC_W)[None, :]).astype(np.float16)
    vones = np.ones((128, NKC * (DK + 1)), np.float16)
    in_maps = []
    for c in range(NCORES):
        b, g = c // HPC, c % HPC
        cols = slice(g * CW, (g + 1) * CW)
        in_maps.append({
            "xt": _round_f32r(x[b].T),
            "wq": _round_f32r(W_Q[:, cols]),
            "wk": _round_f32r(W_K[:, cols]),
            "wv": _round_f32r(W_V[:, cols]),
            "wo": W_O[cols, :].astype(np.float16),
            "ones": ones,
            "tri": tri,
            "vones": vones,
        })
    return in_maps


def gather_output(results):
    out = np.zeros((B, S, D), np.float32)
    for c in range(NCORES):
        out[c // HPC] += results[c]["o0"].T
        out[c // HPC] += results[c]["o1"].T
    return out


def kernel(x, W_Q, W_K, W_V, W_O):
    nc = _get_nc()
    res = run_bass_kernel_spmd(
        nc, make_in_maps(x, W_Q, W_K, W_V, W_O), list(range(NCORES))).results
    return gather_output(res)
